# revision 20
# baseline (speedup 1.0000x reference)
"""Trainium2 Bass kernel for nn_Attention (B=8, L=2048, D=64).

Reference (per batch b):
    BZ = x @ B_w.T + B_b
    CZ = x @ C_w.T + C_b
    scores = BZ @ CZ.T              # (L, L)
    attn = relu(scores)
    attn = attn / (attn.sum(axis=-2, keepdims=True) + EPS)   # column-sum norm
    VZ = x @ V_w.T + V_b
    out = x + attn @ VZ

Strategy (one batch per NeuronCore, 8 cores, no cross-core comms):
  * Host pre-packs x^T column-halves onto the two partition halves
    (xp = [x^T[:, :1024]; x^T[:, 1024:]], fp16) so input DMA moves 2KB
    rows and every matmul operand sits on a clean 64-partition group.
    The device processes l/m in the permuted block order [b0,b2,b1,b3];
    the host un-permutes.  The +x residual is added on the host.
  * No ones-row augmentation: B/C biases fold into the PSUM->SBUF
    projection evacuations (ACT per-partition bias AP / DVE tensor_scalar
    add), the V bias folds into the VZ evacuation as a DVE tensor_tensor
    with a host-replicated V_b tile.
  * Projections run as 64x64 quad-tiled matmuls (4 concurrent PE tiles:
    2 SBUF row groups x 2 PSUM column groups, duplicating BZ^T/CZ^T into
    both partition halves for free); VZ runs row-group-paired.
  * Main loop: S^T chunk scores (row-packed pairs) land in a 3-buffer
    PSUM pool (6 banks) so score matmuls never wait on the previous
    evacuation (the baseline's 2-buffer ping-pong).  The O^T accumulator
    holds the remaining 2 banks.
  * Evacuation split: ACT takes chunk A (Relu + accum colsums), DVE takes
    chunk B (tensor_scalar max + accum).  GpSimd merges the u0/u1 partial
    colsums and scales the VZ rows by the reciprocal; DVE only adds the
    reciprocal itself.  Normalization folds into VZ rows:
    O^T = (VZ*recip)^T @ A^T accumulated in PSUM over m-chunks,
    column-packed into [128, 1024].
  * Output is fp16 (pre-residual values are small) to halve the tail DMA.
  * Warmup filler matmuls bridge the input-DMA wait so the PE clock gate
    opens as early as possible; a dummy activation triggers the ACT
    table load during the DMA wait.
"""

import os
import sys

sys.path.insert(0, "/opt/trn_rl_repo")

import numpy as np

import concourse.bacc as bacc
import concourse.tile as tile
from concourse import mybir
from concourse import bass_utils

L = 2048
D = 64
P = 128
NCH = L // P        # 16 m-chunks
SL = 512            # matmul moving-slice width
EU = 1024           # relu-evacuation unit width (2 PSUM banks)
N_CORES = 8

F32 = mybir.dt.float32
F16 = mybir.dt.float16


def _attention_kernel(ctx, tc, yt_ap, xp_ap, w_ap, wb_ap, cfg):
    nc = tc.nc
    Relu = mybir.ActivationFunctionType.Relu
    Copy = mybir.ActivationFunctionType.Copy
    Ident = mybir.ActivationFunctionType.Identity
    Alu = mybir.AluOpType
    at_dt = F16

    consts = ctx.enter_context(tc.tile_pool(name="consts", bufs=1))
    bigs = ctx.enter_context(tc.tile_pool(name="bigs", bufs=1))
    at_pool = ctx.enter_context(tc.tile_pool(name="at", bufs=7))
    small = ctx.enter_context(tc.tile_pool(name="small", bufs=10))

    # weights on the scalar queue (small, first; keeps ACT free after),
    # all four x pieces on the sync queue in need-order: the scalar queue
    # IS the ACT engine, so parking the big x DMAs there would block the
    # projection evacuations behind ~2.7us of DMA_DIRECT2D issue time.
    w_sb = consts.tile([P, 3 * D], F16)
    nc.scalar.dma_start(out=w_sb, in_=w_ap)
    # biases: col 0 = B_b, col 1 = C_b (dup), cols 2:66 = V_b replicated
    wb_sb = consts.tile([P, 2 + D], F32)
    nc.scalar.dma_start(out=wb_sb, in_=wb_ap)
    bb = wb_sb[:, 0:1]
    cb = wb_sb[:, 1:2]
    vb = wb_sb[:, 2 : 2 + D]

    # x^T column-halves packed on partition halves
    xp = bigs.tile([P, EU], F16)
    nc.sync.dma_start(out=xp[0:D, 0:SL], in_=xp_ap[0:D, 0:SL])
    nc.sync.dma_start(out=xp[D:P, 0:SL], in_=xp_ap[D:P, 0:SL])
    nc.sync.dma_start(out=xp[0:D, SL:EU], in_=xp_ap[0:D, SL:EU])
    nc.sync.dma_start(out=xp[D:P, SL:EU], in_=xp_ap[D:P, SL:EU])

    bz = bigs.tile([P, L], F16)           # BZ^T duplicated on both halves
    cz = bigs.tile([P, L], F16)           # CZ^T duplicated on both halves
    vz_sb = bigs.tile([P, NCH, D], F32)   # VZ natural (m-chunk, d)
    yt_sb = bigs.tile([P, EU], F16)       # O^T col-packed staging

    wu_a = consts.tile([P, SL], F16)
    nc.vector.memset(wu_a, 0.25)
    dummy = small.tile([P, 1], F32, tag="dummy")
    # trigger the ACT table load early, during the DMA wait
    nc.scalar.activation(out=dummy, in_=wu_a[:, 0:1], func=Relu)

    # O^T accumulator lives for the whole kernel: 2 PSUM banks
    po_pool = ctx.enter_context(tc.tile_pool(name="po", bufs=1, space="PSUM"))
    po = po_pool.tile([P, EU], F32)

    # ---------------- prologue ----------------
    # fillers keep the PE clock gate open; they write into the po banks,
    # which the first real O matmul (start=True) later overwrites
    def filler(n):
        for _ in range(n):
            nc.tensor.matmul(po[:, 0:SL], wu_a[:, 0:P], wu_a,
                             start=True, stop=True, skip_group_check=True)

    with tc.tile_pool(name="pp", bufs=3, space="PSUM") as pp_pool:
        filler(cfg["warmup"])

        def proj_piece(w_col, piece):
            # one DMA piece -> 4 quad-tiled matmuls (1 PE window):
            # rows 0-63 of xp = top l-block, rows 64-127 = bottom l-block,
            # each duplicated into both output partition halves.
            pp = pp_pool.tile([P, EU], F32, tag="pp")
            sl = slice(SL * piece, SL * (piece + 1))
            for rg in (0, D):       # SBUF row group (which l-block)
                for cg in (0, D):   # PSUM column group (duplicate)
                    nc.tensor.matmul(
                        pp[cg : cg + D, SL * (rg // D) : SL * (rg // D + 1)],
                        w_sb[rg : rg + D, w_col : w_col + D],
                        xp[rg : rg + D, sl],
                        start=True, stop=True)
            return pp

        def proj_evac(pp, dst, piece, bias_ap, eng):
            # piece 0 holds l-blocks (b0, b2) -> dst cols 0:1024
            # piece 1 holds l-blocks (b1, b3) -> dst cols 1024:2048
            sl = slice(EU * piece, EU * (piece + 1))
            if eng == "act":
                nc.scalar.activation(out=dst[:, sl], in_=pp, func=Ident,
                                     bias=bias_ap)
            else:
                nc.vector.tensor_scalar(out=dst[:, sl], in0=pp,
                                        scalar1=bias_ap, scalar2=None,
                                        op0=Alu.add)

        def vz_round(pvt, piece):
            # 4 row-group-paired window slots: top chunk c || bottom chunk,
            # the pair writing the two different banks of one pp tile
            for c in range(4):
                col = slice(P * c + SL * piece, P * (c + 1) + SL * piece)
                nc.tensor.matmul(pvt[:, 0, piece, c, :],
                                 xp[0:D, col], w_sb[0:D, 2 * D : 3 * D],
                                 start=True, stop=True)
                nc.tensor.matmul(pvt[:, 1, piece, c, :],
                                 xp[D:P, col], w_sb[D:P, 2 * D : 3 * D],
                                 start=True, stop=True)

        # PE order follows the DMA staircase; projections first so their
        # evacuations overlap the VZ matmuls, post-VZ fillers bridge the
        # remaining evacuation latency so the PE never idles (a >1us PE
        # gap re-throttles the clock for ~20us).
        ppb0 = proj_piece(0, 0)                   # B proj, piece 0
        ppc0 = proj_piece(D, 0)                   # C proj, piece 0
        proj_evac(ppb0, bz, 0, bb, "act")
        pvt = pp_pool.tile([P, 2, 2, 4, D], F32, tag="pp")
        vz_round(pvt, 0)                          # VZ chunks 0-7
        ppb1 = proj_piece(0, 1)                   # B proj, piece 1
        proj_evac(ppc0, cz, 0, cb, "act")
        proj_evac(ppb1, bz, 1, bb, "dve")
        vz_round(pvt, 1)                          # VZ chunks 8-15
        ppc1 = proj_piece(D, 1)                   # C proj, piece 1
        proj_evac(ppc1, cz, 1, cb, "act")
        filler(cfg["fill_pro"])
        # VZ evac with V_b add: bank 0 holds chunks {0-3, 8-11} (top row
        # groups), bank 1 holds {4-7, 12-15}; both on DVE (only ACT/DVE
        # reach PSUM, and ACT cannot tensor-add)
        vzv = vz_sb.rearrange("p (b h c) d -> p b h c d", b=2, h=2, c=4)
        vbb = vb.unsqueeze(1).unsqueeze(1).broadcast_to([P, 2, 4, D])
        nc.vector.tensor_tensor(out=vzv[:, :, 0], in0=pvt[:, 0], in1=vbb,
                                op=Alu.add)
        nc.vector.tensor_tensor(out=vzv[:, :, 1], in0=pvt[:, 1], in1=vbb,
                                op=Alu.add)

    # ---------------- main loop ----------------
    def emit_scores(pstile, lo, cc, u):
        for jj in range(2):
            j = 2 * u + jj
            nc.tensor.matmul(pstile[:, SL * jj : SL * (jj + 1)],
                             cz[lo : lo + D, P * cc : P * (cc + 1)],
                             bz[lo : lo + D, SL * j : SL * (j + 1)],
                             start=True, stop=True)

    def emit_o(c, at, vzs, last):
        # column-packed: j 0/1 -> partitions 0-63, j 2/3 -> 64-127
        for j in range(4):
            if j < 2:
                out_ap = po[0:D, SL * j : SL * (j + 1)]
            else:
                out_ap = po[D : 2 * D, SL * (j - 2) : SL * (j - 1)]
            nc.tensor.matmul(out_ap, vzs, at[:, SL * j : SL * (j + 1)],
                             start=(c == 0), stop=last)

    scale_eng = nc.gpsimd if cfg["gps_scale"] else nc.vector
    merge_eng = nc.gpsimd if cfg["gps_merge"] else nc.vector

    def chain_head(pend):
        # reciprocal for a finished pair (one step behind) on DVE
        recip2 = small.tile([P, 2], F32, tag="recip")
        nc.vector.reciprocal(recip2, pend[1])
        return recip2

    def chain_tail(pend, recip2):
        # VZ row scaling on GpSimd
        cA = pend[0]
        vzs2 = small.tile([P, 2, D], at_dt, tag="vzs")
        scale_eng.tensor_tensor(
            out=vzs2, in0=vz_sb[:, cA : cA + 2, :],
            in1=recip2.unsqueeze(2).broadcast_to([P, 2, D]),
            op=Alu.mult)
        return vzs2

    with tc.tile_pool(name="ps", bufs=3, space="PSUM") as ps_pool:
        pend_chain = None     # (cA, csAB, (atA, atB)) awaiting recip
        pend_o = []           # (cA, (atA, atB), vzs2) awaiting O matmuls
        for p in range(NCH // 2):
            cA, cB = 2 * p, 2 * p + 1
            atA = at_pool.tile([P, L], at_dt, tag="at")
            atB = at_pool.tile([P, L], at_dt, tag="at")
            cs4 = small.tile([P, 2, 2], F32, tag="cs4")
            # ---- u0 halves: row-packed scores + evacuation
            psA0 = ps_pool.tile([P, EU], F32, tag="ps")
            emit_scores(psA0, 0, cA, 0)
            psB0 = ps_pool.tile([P, EU], F32, tag="ps")
            emit_scores(psB0, D, cB, 0)
            nc.scalar.activation(out=atA[:, 0:EU], in_=psA0,
                                 func=Relu, accum_out=cs4[:, 0, 0:1])
            nc.vector.tensor_scalar(out=atB[:, 0:EU], in0=psB0,
                                    scalar1=0.0, scalar2=0.0,
                                    op0=Alu.max, op1=Alu.add,
                                    accum_out=cs4[:, 1, 0:1])
            # ---- u1 halves + O matmuls two pairs back
            psA1 = ps_pool.tile([P, EU], F32, tag="ps")
            emit_scores(psA1, 0, cA, 1)
            if pend_chain is not None:
                recip2 = chain_head(pend_chain)
            if p <= 1:
                filler(cfg["fill_p0"])
            psB1 = ps_pool.tile([P, EU], F32, tag="ps")
            emit_scores(psB1, D, cB, 1)
            if len(pend_o) >= 1:
                c0, ats, v0 = pend_o.pop(0)
                emit_o(c0, ats[0], v0[:, 0, :], False)
                emit_o(c0 + 1, ats[1], v0[:, 1, :], False)
            nc.scalar.activation(out=atA[:, EU : 2 * EU], in_=psA1,
                                 func=Relu, accum_out=cs4[:, 0, 1:2])
            nc.vector.tensor_scalar(out=atB[:, EU : 2 * EU], in0=psB1,
                                    scalar1=0.0, scalar2=0.0,
                                    op0=Alu.max, op1=Alu.add,
                                    accum_out=cs4[:, 1, 1:2])
            # ---- den merge + vz scale for the previous pair
            if pend_chain is not None:
                vzs2 = chain_tail(pend_chain, recip2)
                pend_o.append((pend_chain[0], pend_chain[2], vzs2))
            csAB = small.tile([P, 2], F32, tag="csAB")
            merge_eng.tensor_tensor(out=csAB, in0=cs4[:, :, 0],
                                    in1=cs4[:, :, 1], op=Alu.add)
            pend_chain = (cA, csAB, (atA, atB))
        # ---- drain the pipeline
        recip2 = chain_head(pend_chain)
        c0, ats, v0 = pend_o.pop(0)
        emit_o(c0, ats[0], v0[:, 0, :], False)
        emit_o(c0 + 1, ats[1], v0[:, 1, :], False)
        vzs2 = chain_tail(pend_chain, recip2)
        c0, ats = pend_chain[0], pend_chain[2]
        emit_o(c0, ats[0], vzs2[:, 0, :], False)
        emit_o(c0 + 1, ats[1], vzs2[:, 1, :], True)

    # ---------------- epilogue ----------------
    # two parallel half evacs (ACT on po bank 0, DVE on bank 1 -- different
    # banks so they run concurrently), then two parallel output DMAs
    nc.scalar.activation(out=yt_sb[:, 0:SL], in_=po[:, 0:SL], func=Copy)
    nc.vector.tensor_copy(yt_sb[:, SL:EU], po[:, SL:EU])
    nc.sync.dma_start(out=yt_ap[:, 0:SL], in_=yt_sb[:, 0:SL])
    nc.scalar.dma_start(out=yt_ap[:, SL:EU], in_=yt_sb[:, SL:EU])


_CACHE = {}


def _build(gps_scale=True, gps_merge=True, warmup=2, fill_pro=2,
           fill_p0=3):
    key = ("nc", gps_scale, gps_merge, warmup, fill_pro, fill_p0)
    if key in _CACHE:
        return _CACHE[key]
    cfg = {"gps_scale": gps_scale, "gps_merge": gps_merge, "warmup": warmup,
           "fill_pro": fill_pro, "fill_p0": fill_p0}
    nc = bacc.Bacc("TRN2", target_bir_lowering=False, debug=False,
                   enable_asserts=False, num_devices=1)
    xp = nc.dram_tensor("xp", (P, EU), F16, kind="ExternalInput").ap()
    w = nc.dram_tensor("wpack", (P, 3 * D), F16, kind="ExternalInput").ap()
    wb = nc.dram_tensor("wb32", (P, 2 + D), F32, kind="ExternalInput").ap()
    yt = nc.dram_tensor("yt", (P, EU), F16, kind="ExternalOutput").ap()
    from contextlib import ExitStack
    with tile.TileContext(nc) as tc, ExitStack() as ctx:
        _attention_kernel(ctx, tc, yt, xp, w, wb, cfg)
    nc.compile()
    _CACHE[key] = nc
    return nc


# permuted l-block order on device: [b0, b2, b1, b3]
_LPERM = np.concatenate([np.arange(0, 512), np.arange(1024, 1536),
                         np.arange(512, 1024), np.arange(1536, 2048)])


def _pack_weights(B_w, B_b, C_w, C_b, V_w, V_b):
    wt = np.concatenate([np.asarray(B_w, np.float32).T,
                         np.asarray(C_w, np.float32).T,
                         np.asarray(V_w, np.float32).T], axis=1)
    wpack = np.tile(wt.astype(np.float16), (2, 1))          # [128, 192]
    wb32 = np.empty((P, 2 + D), np.float32)
    wb32[:, 0] = np.tile(np.asarray(B_b, np.float32), 2)
    wb32[:, 1] = np.tile(np.asarray(C_b, np.float32), 2)
    wb32[:, 2:] = np.asarray(V_b, np.float32)[None, :]      # replicated
    return wpack, wb32


def run(inputs, trace=False, tmpdir=None, gps_scale=True, gps_merge=True,
        warmup=2, fill_pro=2, fill_p0=3):
    nc = _build(gps_scale, gps_merge, warmup, fill_pro, fill_p0)
    x = np.asarray(inputs["x"], dtype=np.float32)
    wpack, wb32 = _pack_weights(
        inputs["B_w"], inputs["B_b"], inputs["C_w"], inputs["C_b"],
        inputs["V_w"], inputs["V_b"])
    in_maps = []
    for i in range(N_CORES):
        xT = np.ascontiguousarray(x[i].T).astype(np.float16)    # [64, 2048]
        xp = np.concatenate([xT[:, 0:EU], xT[:, EU:L]], axis=0)  # [128, 1024]
        in_maps.append({"xp": xp, "wpack": wpack, "wb32": wb32})
    res = bass_utils.run_bass_kernel_spmd(nc, in_maps,
                                          core_ids=list(range(N_CORES)),
                                          trace=trace, tmpdir=tmpdir)
    out = np.empty((N_CORES, L, D), np.float32)
    for i in range(N_CORES):
        yt = res.results[i]["yt"].astype(np.float32)
        ot = np.concatenate([yt[0:D, :], yt[D:P, :]], axis=1)   # [64, 2048]
        out[i, _LPERM, :] = ot.T
    out += x
    return out, res


def kernel(**inputs) -> np.ndarray:
    out, _ = run(inputs, trace=False)
    return out
